# revision 4
# baseline (speedup 1.0000x reference)
"""Trainium2 Bass kernel for nn_ConvSelfAttentionModule (B=4, C=256, H=W=64).

Reference computation per image (xf = x reshaped to [C, N], N = H*W = 4096):
    q = wq @ xf + bq                       [128, N]
    k = wk @ xf + bk                       [128, N]
    v = wv @ xf + bv                       [256, N]
    s[m, n]   = sum_d q[d, m] k[d, n]      [N, N]
    attn      = softmax_n(s)
    af[c, n]  = sum_m v[c, m] attn[m, n]   [256, N]
    out = gamma * af + x

Sharding: 8 cores = 4 images x 2 m-chunks of M=2048 rows of the attention
matrix. Core (b, half) computes af_part[c, n] = sum_{m in chunk} v'[c, m] E[m, n]
for all n, where E = exp(s - 20) and v'[c, m] = gamma * v[c, m] / rowsum_E[m].
The host adds the two partials per image and adds x. Each core receives its
image's xf pre-rolled by -m0 columns so the kernel is SPMD-identical.

v2 design notes:
  - Everything bf16 on the PE: host ships x / weights in bf16, q/k are
    evacuated to bf16, scores + apply matmuls run bf16 (full PE rate, FWL
    weight loads). exp(s-20) reads the f32 PSUM scores directly.
  - The ACT (scalar) engine is the hard wall: 8.4M exps at 1 elem/lane/cycle
    @1.2GHz ~= 60us/core. Everything else is scheduled to keep ACT busy on
    exp: all other PSUM evacuations ride DVE or happen before the first exp.
  - Rowsums via DVE reduce_sum over the bf16 E tiles (not ACT accum_out,
    which costs extra ACTIVATION_READ_ACCUMULATOR slots on ACT).
  - PSUM: one pool, tags 'ps' (2 x [128,1024] score slots, rotating) and
    'wg' (1 x [128,2048]): 8 banks exactly. 'wg' hosts the k-group-1 and v
    projections during the h0 sweep, then becomes the (h0,c0) attention-apply
    quarter accumulated in the PE idle under the h1 exp sweep.
  - Remaining 6 apply chunks of [128,1024] rotate through the 'ps' slots,
    evacuated to bf16 and DMA'd out as they finish.
"""

import numpy as np
import ml_dtypes

import concourse.bass as bass  # noqa: F401  (bass types via bacc/tile)
import concourse.tile as tile
from concourse import bacc, mybir
from concourse.bass_utils import run_bass_kernel_spmd

dt = mybir.dt
bf16_np = ml_dtypes.bfloat16

P = 128          # partitions / q,k channel dim
C = 256          # channels
N = 4096         # pixels per image
M = 2048         # per-core m-chunk
MT = M // P      # 16 m-tiles
B = 4
N_CORES = 8
EXP_SHIFT = -20.0  # constant subtracted inside exp; cancels in softmax

_CACHE = {}


def build_nc():
    nc = bacc.Bacc("TRN2", target_bir_lowering=False, debug=False,
                   num_devices=N_CORES)
    f32, bf16 = dt.float32, dt.bfloat16
    AF = mybir.ActivationFunctionType
    AX = mybir.AxisListType.X

    x = nc.dram_tensor("x", [C, N], bf16, kind="ExternalInput").ap()
    wqT = nc.dram_tensor("wqT", [C, P], bf16, kind="ExternalInput").ap()
    wkT = nc.dram_tensor("wkT", [C, P], bf16, kind="ExternalInput").ap()
    wvT = nc.dram_tensor("wvT", [C, C], bf16, kind="ExternalInput").ap()
    bq = nc.dram_tensor("bq", [P, 1], f32, kind="ExternalInput").ap()
    bk = nc.dram_tensor("bk", [P, 1], f32, kind="ExternalInput").ap()
    bvr = nc.dram_tensor("bvr", [1, C], f32, kind="ExternalInput").ap()
    out = nc.dram_tensor("out_part", [C, N], bf16, kind="ExternalOutput").ap()

    with tile.TileContext(nc) as tc:
        with (
            tc.tile_pool(name="consts", bufs=1) as consts,
            tc.tile_pool(name="xs", bufs=1) as xs,
            tc.tile_pool(name="big", bufs=1) as big,
            tc.tile_pool(name="es", bufs=MT) as es,
            tc.tile_pool(name="afs", bufs=4) as afs,
            tc.tile_pool(name="ps", bufs=2, space="PSUM") as ps,
        ):
            # ---- weights / biases.  wk, wq ride the sync queue (HWDGE,
            # ~0.6us issue) interleaved with the x chunks they gate; the
            # rest go SWDGE on gpsimd off the critical path. ----
            wk_t, wq_t, wv_t = [], [], []
            for i in range(2):
                wki = consts.tile([P, P], bf16, name=f"wk{i}", tag=f"wk{i}")
                nc.sync.dma_start(out=wki, in_=wkT[i * P:(i + 1) * P, :])
                wk_t.append(wki)

            # x group 0 (cols 0:2048) as 2 chunk-pairs of 1024 on sync.
            x0 = xs.tile([P, M], bf16, name="xg0_0", tag="xg0_0")
            x1 = xs.tile([P, M], bf16, name="xg1_0", tag="xg1_0")
            nc.sync.dma_start(out=x0[:, 0:1024], in_=x[0:P, 0:1024])
            nc.sync.dma_start(out=x1[:, 0:1024], in_=x[P:C, 0:1024])
            for i in range(2):
                wqi = consts.tile([P, P], bf16, name=f"wq{i}", tag=f"wq{i}")
                nc.sync.dma_start(out=wqi, in_=wqT[i * P:(i + 1) * P, :])
                wq_t.append(wqi)
            nc.sync.dma_start(out=x0[:, 1024:2048], in_=x[0:P, 1024:2048])
            nc.sync.dma_start(out=x1[:, 1024:2048], in_=x[P:C, 1024:2048])

            # x group 1 (cols 2048:4096) on the scalar queue (ACT is idle
            # during the prologue; these issues cost ~1.2us before the
            # first k0 evacuation, which waits on matmuls anyway).
            x2 = xs.tile([P, M], bf16, name="xg0_1", tag="xg0_1")
            x3 = xs.tile([P, M], bf16, name="xg1_1", tag="xg1_1")
            nc.scalar.dma_start(out=x2, in_=x[0:P, M:N])
            nc.scalar.dma_start(out=x3, in_=x[P:C, M:N])

            bk_t = consts.tile([P, 1], f32, name="bk_t", tag="bk_t")
            nc.gpsimd.dma_start(out=bk_t, in_=bk)
            bq_t = consts.tile([P, 1], f32, name="bq_t", tag="bq_t")
            nc.gpsimd.dma_start(out=bq_t, in_=bq)
            for i in range(2):
                wvi = consts.tile([P, C], bf16, name=f"wv{i}", tag=f"wv{i}")
                nc.gpsimd.dma_start(out=wvi, in_=wvT[i * P:(i + 1) * P, :])
                wv_t.append(wvi)
            bv_bc = consts.tile([P, C], f32, name="bv_bc", tag="bv_bc")
            nc.gpsimd.dma_start(out=bv_bc, in_=bvr.to_broadcast((P, C)))
            shift_t = consts.tile([P, 1], f32, name="shift_t", tag="shift_t")
            nc.vector.memset(shift_t, EXP_SHIFT)

            # Dummy exp so the ACT function-table load (~2.7us) happens
            # during the DMA prologue, not before the first real exp.
            warm_t = consts.tile([P, 1], f32, name="warm_t", tag="warm_t")
            nc.scalar.activation(warm_t, shift_t, AF.Exp, bias=shift_t[:, 0:1],
                                 scale=1.0)

            rs = consts.tile([P, MT, 4], f32, name="rs", tag="rs")
            rr = consts.tile([P, MT], f32, name="rr", tag="rr")

            k_sb = big.tile([P, N], bf16, name="k_sb", tag="k_sb")
            q_sb = big.tile([P, M], bf16, name="q_sb", tag="q_sb")
            v_sb = big.tile([P, MT, C], bf16, name="v_sb", tag="v_sb")

            # ---- prologue: k group 0 + q, ACT-evacuated (with bias) in
            # 1024-wide passes chasing the x chunks.  These gate exp0. ----
            for h in range(2):  # 1024-col halves of k0
                kp = ps.tile([P, 1024], f32, name=f"kp0_{h}", tag="ps")
                for j in range(2):
                    sl = slice(h * 1024 + j * 512, h * 1024 + (j + 1) * 512)
                    psl = slice(j * 512, (j + 1) * 512)
                    nc.tensor.matmul(kp[:, psl], wk_t[0], x0[:, sl],
                                     start=True, stop=False)
                    nc.tensor.matmul(kp[:, psl], wk_t[1], x1[:, sl],
                                     start=False, stop=True)
                nc.scalar.activation(k_sb[:, h * 1024:(h + 1) * 1024], kp,
                                     AF.Identity, bias=bk_t[:, 0:1], scale=1.0)
            for h in range(2):  # 1024-col halves of q
                qp = ps.tile([P, 1024], f32, name=f"qp_{h}", tag="ps")
                for j in range(2):
                    sl = slice(h * 1024 + j * 512, h * 1024 + (j + 1) * 512)
                    psl = slice(j * 512, (j + 1) * 512)
                    nc.tensor.matmul(qp[:, psl], wq_t[0], x0[:, sl],
                                     start=True, stop=False)
                    nc.tensor.matmul(qp[:, psl], wq_t[1], x1[:, sl],
                                     start=False, stop=True)
                nc.scalar.activation(q_sb[:, h * 1024:(h + 1) * 1024], qp,
                                     AF.Identity, bias=bq_t[:, 0:1], scale=1.0)

            # ---- deferred projections: issued inside the h0 sweep, run in
            # the PE idle under the exp stream, DVE-evacuated, in the 'wg'
            # PSUM slot so the score slots stay double-buffered. ----
            def k1_pass():
                kp = ps.tile([P, M], f32, name="kp1", tag="wg", bufs=1)
                for j in range(4):
                    sl = slice(j * 512, (j + 1) * 512)
                    nc.tensor.matmul(kp[:, sl], wk_t[0], x2[:, sl],
                                     start=True, stop=False)
                    nc.tensor.matmul(kp[:, sl], wk_t[1], x3[:, sl],
                                     start=False, stop=True)
                nc.vector.tensor_scalar_add(k_sb[:, M:2 * M], kp,
                                            bk_t[:, 0:1])

            def v_pass(p):
                # 8 m-tiles of vT per pass, packed [128, 8*256] in 'wg'
                vp = ps.tile([P, M], f32, name=f"vp{p}", tag="wg", bufs=1)
                for i in range(8):
                    t = p * 8 + i
                    xsl = slice(t * P, (t + 1) * P)
                    vsl = slice(i * C, (i + 1) * C)
                    nc.tensor.matmul(vp[:, vsl], x0[:, xsl], wv_t[0],
                                     start=True, stop=False)
                    nc.tensor.matmul(vp[:, vsl], x1[:, xsl], wv_t[1],
                                     start=False, stop=True)
                for i in range(8):
                    t = p * 8 + i
                    nc.vector.tensor_add(v_sb[:, t, :],
                                         vp[:, i * C:(i + 1) * C], bv_bc)

            e_tiles = [es.tile([P, 2, M], bf16, name=f"e{mt}", tag="e")
                       for mt in range(MT)]

            def scores_tile(mt, h, half):
                """One [128,1024] score tile: cols h*2048 + half*1024 ...,
                exp'd to E bf16, rowsum to rs[:, mt, 2*h+half]."""
                q_l = q_sb[:, mt * P:(mt + 1) * P]
                sp = ps.tile([P, 1024], f32, name=f"sp{mt}_{h}{half}",
                             tag="ps")
                base = h * M + half * 1024
                for j in range(2):
                    k_l = k_sb[:, base + j * 512:base + (j + 1) * 512]
                    nc.tensor.matmul(sp[:, j * 512:(j + 1) * 512], q_l,
                                     k_l, start=True, stop=True)
                esl = slice(half * 1024, (half + 1) * 1024)
                nc.scalar.activation(e_tiles[mt][:, h, esl], sp, AF.Exp,
                                     bias=shift_t[:, 0:1], scale=1.0)
                nc.vector.reduce_sum(rs[:, mt, 2 * h + half:2 * h + half + 1],
                                     e_tiles[mt][:, h, esl], axis=AX)

            # ---- h0 sweep, with k1 + v passes wedged in the PE idle ----
            for mt in range(MT):
                scores_tile(mt, 0, 0)
                scores_tile(mt, 0, 1)
                if mt == 0:
                    k1_pass()
                elif mt == 2:
                    v_pass(0)
                elif mt == 5:
                    v_pass(1)

            # ---- h1 sweep + softmax fold + wedged apply quarter (h0, c0)
            # accumulating in 'wg' under the exp stream ----
            af_q = ps.tile([P, M], f32, name="af_q", tag="wg", bufs=1)

            def fold(mt):
                nc.vector.reduce_sum(rr[:, mt:mt + 1], rs[:, mt, :], axis=AX)
                nc.vector.reciprocal(rr[:, mt:mt + 1], rr[:, mt:mt + 1])
                nc.vector.tensor_scalar_mul(v_sb[:, mt, :], v_sb[:, mt, :],
                                            rr[:, mt:mt + 1])

            def af_q_mm(mt):
                lhs = v_sb[:, mt, 0:P]
                for j in range(4):
                    nc.tensor.matmul(af_q[:, j * 512:(j + 1) * 512], lhs,
                                     e_tiles[mt][:, 0, j * 512:(j + 1) * 512],
                                     start=(mt == 0), stop=(mt == MT - 1))

            for mt in range(MT):
                scores_tile(mt, 1, 0)
                scores_tile(mt, 1, 1)
                fold(mt)
                if mt >= 1:
                    af_q_mm(mt - 1)
            af_q_mm(MT - 1)

            # ---- apply: evac the wedged quarter, then 6 remaining
            # [128,1024] chunks rotating through the 'ps' slots ----
            def store(src_ps, h, c, half, engine):
                af_t = afs.tile([P, 1024], bf16, name=f"af{h}{c}{half}",
                                tag="af")
                if engine == "scalar":
                    nc.scalar.copy(af_t, src_ps)
                else:
                    nc.vector.tensor_copy(af_t, src_ps)
                nc.sync.dma_start(
                    out=out[c * P:(c + 1) * P,
                            h * M + half * 1024:h * M + (half + 1) * 1024],
                    in_=af_t)

            # wedged quarter (h0, c0) is complete: evac both halves on ACT
            # (idle now) while the PE starts the next chunks.
            for half in range(2):
                store(af_q[:, half * 1024:(half + 1) * 1024], 0, 0, half,
                      "scalar")

            rest = [(0, 1, 0), (0, 1, 1), (1, 0, 0), (1, 0, 1),
                    (1, 1, 0), (1, 1, 1)]
            for idx, (h, c, half) in enumerate(rest):
                ap_ps = ps.tile([P, 1024], f32, name=f"afp{h}{c}{half}",
                                tag="ps")
                for mt in range(MT):
                    lhs = v_sb[:, mt, c * P:(c + 1) * P]
                    base = half * 1024
                    for j in range(2):
                        nc.tensor.matmul(
                            ap_ps[:, j * 512:(j + 1) * 512], lhs,
                            e_tiles[mt][:, h,
                                        base + j * 512:base + (j + 1) * 512],
                            start=(mt == 0), stop=(mt == MT - 1))
                store(ap_ps, h, c, half,
                      "scalar" if idx % 2 == 0 else "vector")

    nc.compile()
    return nc


def _get_nc():
    if "nc" not in _CACHE:
        _CACHE["nc"] = build_nc()
    return _CACHE["nc"]


def build_in_maps(x, wq, bq, wk, bk, wv, bv, gamma):
    x = np.asarray(x, np.float32)
    g = float(np.asarray(gamma).reshape(-1)[0])
    wqT = np.ascontiguousarray(np.asarray(wq, np.float32).T).astype(bf16_np)
    wkT = np.ascontiguousarray(np.asarray(wk, np.float32).T).astype(bf16_np)
    wvT = np.ascontiguousarray(
        (g * np.asarray(wv, np.float32)).T).astype(bf16_np)
    bq2 = np.ascontiguousarray(np.asarray(bq, np.float32).reshape(P, 1))
    bk2 = np.ascontiguousarray(np.asarray(bk, np.float32).reshape(P, 1))
    bvr = np.ascontiguousarray((g * np.asarray(bv, np.float32)).reshape(1, C))
    xf = x.reshape(B, C, N)
    in_maps = []
    for core in range(N_CORES):
        b, half = core // 2, core % 2
        xc = xf[b] if half == 0 else np.roll(xf[b], -M, axis=1)
        in_maps.append(dict(x=np.ascontiguousarray(xc).astype(bf16_np),
                            wqT=wqT, wkT=wkT, wvT=wvT,
                            bq=bq2, bk=bk2, bvr=bvr))
    return in_maps


def assemble(results, x):
    x = np.asarray(x, np.float32)
    af = np.zeros((B, C, N), np.float32)
    for core in range(N_CORES):
        b, half = core // 2, core % 2
        part = np.asarray(results[core]["out_part"], dtype=np.float32)
        af[b] += part if half == 0 else np.roll(part, M, axis=1)
    return (af.reshape(x.shape) + x).astype(np.float32)


def kernel(x, wq, bq, wk, bk, wv, bv, gamma):
    nc = _get_nc()
    in_maps = build_in_maps(x, wq, bq, wk, bk, wv, bv, gamma)
    res = run_bass_kernel_spmd(nc, in_maps, core_ids=list(range(N_CORES)))
    return assemble(res.results, x)


# revision 7
# speedup vs baseline: 1.1152x; 1.1152x over previous
"""Trainium2 Bass kernel for nn_ConvSelfAttentionModule (B=4, C=256, H=W=64).

Reference computation per image (xf = x reshaped to [C, N], N = H*W = 4096):
    q = wq @ xf + bq                       [128, N]
    k = wk @ xf + bk                       [128, N]
    v = wv @ xf + bv                       [256, N]
    s[m, n]   = sum_d q[d, m] k[d, n]      [N, N]
    attn      = softmax_n(s)
    af[c, n]  = sum_m v[c, m] attn[m, n]   [256, N]
    out = gamma * af + x

Sharding: 8 cores = 4 images x 2 m-chunks of M=2048 rows of the attention
matrix. Core (b, half) computes af_part[c, n] = sum_{m in chunk} v'[c, m] E[m, n]
for all n, where E = exp(s - 20) and v'[c, m] = gamma * v[c, m] / rowsum_E[m].
The host adds the two partials per image and adds x. Each core receives its
image's xf pre-rolled by -m0 columns so the kernel is SPMD-identical.

v3 design (engine-level):
  - All matmuls bf16 (host ships x / weights bf16): full PE rate + FWL.
  - ACT is the hard wall: 8.4M exps ~= 59us/core at W=2048 per ACTIVATE
    (back-to-back spacing ~1850ns measured; W=1024 costs +30% in per-inst
    overhead).  W=2048 f32 score tiles double-buffered = all 8 PSUM banks,
    which is why nothing else can overlap the exp sweeps in PSUM.
  - ACT does ONLY: k0 evacuation (prologue, before exp0) + 32 exps + half
    the apply evacuations (post-sweep).  Everything else is DVE/GPSIMD:
    q/k1/v evacuations on DVE (tensor_scalar_add / tensor_add with
    materialized broadcast biases), rowsums split gpsimd (h0) / DVE-gpsimd
    (h1 alternating) as reduce_sum over the bf16 E tiles in SBUF.
  - Single fused sweep, mt-major (h0 then h1 per mt) so the softmax fold
    for row-block mt completes during the sweep.  k-group-1 and the two
    v-projection passes ride spare PSUM slot rotations early in the sweep.
  - Apply: 4 quarters [128,2048] rotating the 2 PSUM slots, evac alternating
    ACT/DVE, DMA out per 1024-wide half as soon as evacuated.
"""

import numpy as np
import ml_dtypes

import concourse.bass as bass  # noqa: F401  (bass types via bacc/tile)
import concourse.tile as tile
from concourse import bacc, mybir
from concourse.bass_utils import run_bass_kernel_spmd

dt = mybir.dt
bf16_np = ml_dtypes.bfloat16

P = 128          # partitions / q,k channel dim
C = 256          # channels
N = 4096         # pixels per image
M = 2048         # per-core m-chunk
MT = M // P      # 16 m-tiles
B = 4
N_CORES = 8
EXP_SHIFT = -20.0  # constant subtracted inside exp; cancels in softmax

_CACHE = {}


def build_nc():
    nc = bacc.Bacc("TRN2", target_bir_lowering=False, debug=False,
                   num_devices=N_CORES)
    f32, bf16 = dt.float32, dt.bfloat16
    AF = mybir.ActivationFunctionType
    AX = mybir.AxisListType.X

    x = nc.dram_tensor("x", [C, N], bf16, kind="ExternalInput").ap()
    wqT = nc.dram_tensor("wqT", [C, P], bf16, kind="ExternalInput").ap()
    wkT = nc.dram_tensor("wkT", [C, P], bf16, kind="ExternalInput").ap()
    wvT = nc.dram_tensor("wvT", [C, C], bf16, kind="ExternalInput").ap()
    bq = nc.dram_tensor("bq", [P, 1], f32, kind="ExternalInput").ap()
    bk = nc.dram_tensor("bk", [P, 1], f32, kind="ExternalInput").ap()
    bvr = nc.dram_tensor("bvr", [1, C], f32, kind="ExternalInput").ap()
    out = nc.dram_tensor("out_part", [C, N], bf16, kind="ExternalOutput").ap()

    with tile.TileContext(nc) as tc:
        with (
            tc.tile_pool(name="consts", bufs=1) as consts,
            tc.tile_pool(name="xs", bufs=1) as xs,
            tc.tile_pool(name="big", bufs=1) as big,
            tc.tile_pool(name="es", bufs=MT) as es,
            tc.tile_pool(name="afs", bufs=4) as afs,
            tc.tile_pool(name="ps", bufs=2, space="PSUM") as ps,
        ):
            # ---- DMAs.  Sync queue carries what gates exp0, interleaved so
            # the prologue matmuls chase the transfers. ----
            wk_t, wq_t, wv_t = [], [], []
            for i in range(2):
                wki = consts.tile([P, P], bf16, name=f"wk{i}", tag=f"wk{i}")
                nc.sync.dma_start(out=wki, in_=wkT[i * P:(i + 1) * P, :])
                wk_t.append(wki)
            x0 = xs.tile([P, M], bf16, name="x0", tag="x0")
            x1 = xs.tile([P, M], bf16, name="x1", tag="x1")
            nc.sync.dma_start(out=x0[:, 0:1024], in_=x[0:P, 0:1024])
            nc.sync.dma_start(out=x1[:, 0:1024], in_=x[P:C, 0:1024])
            for i in range(2):
                wqi = consts.tile([P, P], bf16, name=f"wq{i}", tag=f"wq{i}")
                nc.sync.dma_start(out=wqi, in_=wqT[i * P:(i + 1) * P, :])
                wq_t.append(wqi)
            nc.sync.dma_start(out=x0[:, 1024:2048], in_=x[0:P, 1024:2048])
            nc.sync.dma_start(out=x1[:, 1024:2048], in_=x[P:C, 1024:2048])

            # x group 1 (cols 2048:4096, feeds k1 only) on the scalar queue;
            # issued before any ACT compute.
            x2 = xs.tile([P, M], bf16, name="x2", tag="x2")
            x3 = xs.tile([P, M], bf16, name="x3", tag="x3")
            nc.scalar.dma_start(out=x2, in_=x[0:P, M:N])
            nc.scalar.dma_start(out=x3, in_=x[P:C, M:N])

            bk_t = consts.tile([P, 1], f32, name="bk_t", tag="bk_t")
            nc.gpsimd.dma_start(out=bk_t, in_=bk)
            bq_t = consts.tile([P, 1], f32, name="bq_t", tag="bq_t")
            nc.gpsimd.dma_start(out=bq_t, in_=bq)
            for i in range(2):
                wvi = consts.tile([P, C], bf16, name=f"wv{i}", tag=f"wv{i}")
                nc.gpsimd.dma_start(out=wvi, in_=wvT[i * P:(i + 1) * P, :])
                wv_t.append(wvi)
            bv_bc = consts.tile([P, C], f32, name="bv_bc", tag="bv_bc")
            nc.gpsimd.dma_start(out=bv_bc, in_=bvr.to_broadcast((P, C)))
            shift_t = consts.tile([P, 1], f32, name="shift_t", tag="shift_t")
            nc.vector.memset(shift_t, EXP_SHIFT)

            # Dummy exp so the ACT function-table load (~2.7us) happens
            # during the DMA prologue, not before the first real exp.
            warm_t = consts.tile([P, 1], f32, name="warm_t", tag="warm_t")
            nc.scalar.activation(warm_t, shift_t, AF.Exp, bias=shift_t[:, 0:1],
                                 scale=1.0)

            rs = consts.tile([P, MT, 2], f32, name="rs", tag="rs")
            rr = consts.tile([P, MT], f32, name="rr", tag="rr")

            k_sb = big.tile([P, N], bf16, name="k_sb", tag="k_sb")
            q_sb = big.tile([P, M], bf16, name="q_sb", tag="q_sb")
            v_sb = big.tile([P, MT, C], bf16, name="v_sb", tag="v_sb")

            # ---- prologue: k0 (ACT evac w/ bias, gates exp0) and q (DVE
            # evac), both chunk-chasing the x halves. ----
            kp = ps.tile([P, M], f32, name="kp", tag="ps")
            for h in range(2):
                for j in range(2):
                    sl = slice(h * 1024 + j * 512, h * 1024 + (j + 1) * 512)
                    nc.tensor.matmul(kp[:, sl], wk_t[0], x0[:, sl],
                                     start=True, stop=False)
                    nc.tensor.matmul(kp[:, sl], wk_t[1], x1[:, sl],
                                     start=False, stop=True)
                hs = slice(h * 1024, (h + 1) * 1024)
                nc.scalar.activation(k_sb[:, hs], kp[:, hs], AF.Identity,
                                     bias=bk_t[:, 0:1], scale=1.0)
            qp = ps.tile([P, M], f32, name="qp", tag="ps")
            for h in range(2):
                for j in range(2):
                    sl = slice(h * 1024 + j * 512, h * 1024 + (j + 1) * 512)
                    nc.tensor.matmul(qp[:, sl], wq_t[0], x0[:, sl],
                                     start=True, stop=False)
                    nc.tensor.matmul(qp[:, sl], wq_t[1], x1[:, sl],
                                     start=False, stop=True)
                hs = slice(h * 1024, (h + 1) * 1024)
                nc.vector.tensor_scalar_add(q_sb[:, hs], qp[:, hs],
                                            bq_t[:, 0:1])

            # ---- deferred projections, each one rotation of a PSUM slot,
            # wedged early in the sweep (PE idles under the exp stream) ----
            def k1_pass():
                kp1 = ps.tile([P, M], f32, name="kp1", tag="ps")
                for j in range(4):
                    sl = slice(j * 512, (j + 1) * 512)
                    nc.tensor.matmul(kp1[:, sl], wk_t[0], x2[:, sl],
                                     start=True, stop=False)
                    nc.tensor.matmul(kp1[:, sl], wk_t[1], x3[:, sl],
                                     start=False, stop=True)
                nc.vector.tensor_scalar_add(k_sb[:, M:2 * M], kp1,
                                            bk_t[:, 0:1])

            def v_pass(p):
                # 8 m-tiles of vT per pass, packed [128, 8*256]
                vp = ps.tile([P, M], f32, name=f"vp{p}", tag="ps")
                for i in range(8):
                    t = p * 8 + i
                    xsl = slice(t * P, (t + 1) * P)
                    vsl = slice(i * C, (i + 1) * C)
                    nc.tensor.matmul(vp[:, vsl], x0[:, xsl], wv_t[0],
                                     start=True, stop=False)
                    nc.tensor.matmul(vp[:, vsl], x1[:, xsl], wv_t[1],
                                     start=False, stop=True)
                for i in range(8):
                    t = p * 8 + i
                    nc.vector.tensor_add(v_sb[:, t, :],
                                         vp[:, i * C:(i + 1) * C], bv_bc)

            e_tiles = [es.tile([P, 2, M], bf16, name=f"e{mt}", tag="e")
                       for mt in range(MT)]

            def scores_step(mt, h):
                """[128,2048] scores for (mt, h): 4 matmuls, one W=2048 exp
                whose accum_out yields the rowsum for free (well, ~284ns of
                ACTIVATION_READ_ACCUMULATOR — still the cheapest option:
                GPSIMD can't reduce the free axis and DVE reduces would
                saturate that engine)."""
                q_l = q_sb[:, mt * P:(mt + 1) * P]
                sp = ps.tile([P, M], f32, name=f"sp{mt}_{h}", tag="ps")
                for j in range(4):
                    k_l = k_sb[:, h * M + j * 512:h * M + (j + 1) * 512]
                    nc.tensor.matmul(sp[:, j * 512:(j + 1) * 512], q_l,
                                     k_l, start=True, stop=True)
                nc.scalar.activation(e_tiles[mt][:, h, :], sp, AF.Exp,
                                     bias=shift_t[:, 0:1], scale=1.0,
                                     accum_out=rs[:, mt, h:h + 1])

            def fold(mt):
                nc.vector.reduce_sum(rr[:, mt:mt + 1], rs[:, mt, :], axis=AX)
                nc.vector.reciprocal(rr[:, mt:mt + 1], rr[:, mt:mt + 1])
                nc.vector.tensor_scalar_mul(v_sb[:, mt, :], v_sb[:, mt, :],
                                            rr[:, mt:mt + 1])

            # ---- h0 sweep (k1/v wedged: k1 must precede any h1 scores,
            # v passes must precede the folds in the h1 sweep) ----
            for mt in range(MT):
                scores_step(mt, 0)
                if mt == 1:
                    k1_pass()
                elif mt == 3:
                    v_pass(0)
                elif mt == 5:
                    v_pass(1)
            # ---- h1 sweep + softmax fold per mt ----
            for mt in range(MT):
                scores_step(mt, 1)
                fold(mt)

            # ---- apply: 4 quarters [128,2048] rotating the 2 slots ----
            def store(src_ps, h, c, half, engine):
                af_t = afs.tile([P, 1024], bf16, name=f"af{h}{c}{half}",
                                tag="af")
                if engine == "scalar":
                    nc.scalar.copy(af_t, src_ps)
                else:
                    nc.vector.tensor_copy(af_t, src_ps)
                nc.sync.dma_start(
                    out=out[c * P:(c + 1) * P,
                            h * M + half * 1024:h * M + (half + 1) * 1024],
                    in_=af_t)

            quarters = [(0, 0), (0, 1), (1, 0), (1, 1)]
            for h, c in quarters:
                ap_ps = ps.tile([P, M], f32, name=f"afp{h}{c}", tag="ps")
                for mt in range(MT):
                    lhs = v_sb[:, mt, c * P:(c + 1) * P]
                    for j in range(4):
                        nc.tensor.matmul(
                            ap_ps[:, j * 512:(j + 1) * 512], lhs,
                            e_tiles[mt][:, h, j * 512:(j + 1) * 512],
                            start=(mt == 0), stop=(mt == MT - 1))
                for half in range(2):
                    store(ap_ps[:, half * 1024:(half + 1) * 1024], h, c, half,
                          "scalar" if half == 0 else "vector")

    nc.compile()
    return nc


def _get_nc():
    if "nc" not in _CACHE:
        _CACHE["nc"] = build_nc()
    return _CACHE["nc"]


def build_in_maps(x, wq, bq, wk, bk, wv, bv, gamma):
    x = np.asarray(x, np.float32)
    g = float(np.asarray(gamma).reshape(-1)[0])
    wqT = np.ascontiguousarray(np.asarray(wq, np.float32).T).astype(bf16_np)
    wkT = np.ascontiguousarray(np.asarray(wk, np.float32).T).astype(bf16_np)
    wvT = np.ascontiguousarray(
        (g * np.asarray(wv, np.float32)).T).astype(bf16_np)
    bq2 = np.ascontiguousarray(np.asarray(bq, np.float32).reshape(P, 1))
    bk2 = np.ascontiguousarray(np.asarray(bk, np.float32).reshape(P, 1))
    bvr = np.ascontiguousarray((g * np.asarray(bv, np.float32)).reshape(1, C))
    xf = x.reshape(B, C, N)
    in_maps = []
    for core in range(N_CORES):
        b, half = core // 2, core % 2
        xc = xf[b] if half == 0 else np.roll(xf[b], -M, axis=1)
        in_maps.append(dict(x=np.ascontiguousarray(xc).astype(bf16_np),
                            wqT=wqT, wkT=wkT, wvT=wvT,
                            bq=bq2, bk=bk2, bvr=bvr))
    return in_maps


def assemble(results, x):
    x = np.asarray(x, np.float32)
    af = np.zeros((B, C, N), np.float32)
    for core in range(N_CORES):
        b, half = core // 2, core % 2
        part = np.asarray(results[core]["out_part"], dtype=np.float32)
        af[b] += part if half == 0 else np.roll(part, M, axis=1)
    return (af.reshape(x.shape) + x).astype(np.float32)


def kernel(x, wq, bq, wk, bk, wv, bv, gamma):
    nc = _get_nc()
    in_maps = build_in_maps(x, wq, bq, wk, bk, wv, bv, gamma)
    res = run_bass_kernel_spmd(nc, in_maps, core_ids=list(range(N_CORES)))
    return assemble(res.results, x)
